# revision 14
# baseline (speedup 1.0000x reference)
"""Compressed MoE block on 8 Trainium2 NeuronCores.

Expert-parallel sharding: core e owns expert e. The router (tiny: T x H @
H x E) runs on host as part of dispatch; tokens are gathered per selected
expert (top-2), padded to a fixed capacity, and each core runs the full
factored FFN chain for its expert in token-transposed layout:

    g1T = Ug'(e).T @ xT          (Ug' = Ug @ Cg folded on host)
    gT  = Vg(e).T  @ g1T
    u1T = Uu'(e).T @ xT
    uT  = Vu(e).T  @ u1T
    aT  = silu(gT) * uT
    d1T = Ud'(e).T @ aT          (Ud' = Ud @ Cd)
    yT  = Vd(e).T  @ d1T

Everything on-chip is bf16 (PSUM accumulation fp32): halves HBM traffic
vs fp32 while staying well inside the accuracy budget. Weights stream in
consumption order on one DMA ring. Phase B is software-pipelined: the
down-proj (d1) matmuls for f-tile f are issued during f+1's gate/up
matmuls so the silu*up vector-engine latency never stalls the PE. Output
is staged to SBUF bf16 and written back in four 2-m-tile DMAs on
rotating queues so the drain overlaps phase C.
"""

import numpy as np
import ml_dtypes

import concourse.bacc as bacc
import concourse.mybir as mybir
import concourse.tile as tile
from concourse.bass_utils import run_bass_kernel_spmd

F32 = mybir.dt.float32
BF16 = mybir.dt.bfloat16
BF = ml_dtypes.bfloat16

E = 8
KTOP = 2
H = 1024
FF = 2816
R = 256
KH = H // 128    # 8
KR = R // 128    # 2
KF = FF // 128   # 22
MH = H // 128    # 8

_BUILD_CACHE = {}
LAST_RESULT = None


def _build(C, nch):
    """Build the per-core bass program for capacity C split into nch chunks."""
    chunk = C // nch
    AB = 2 * R + C      # per-k block in abuf: [ugc_k | uuc_k | xt_k]
    WB = 3 * R          # per-f block in wbuf: [vg_f | vu_f | udc_f]
    nc = bacc.Bacc()

    abuf = nc.declare_dram_parameter("abuf", [128, KH * AB], BF16, isOutput=False)
    wbuf = nc.declare_dram_parameter("wbuf", [128, KF * WB], BF16, isOutput=False)
    vdp = nc.declare_dram_parameter("vdp", [128, MH * R], BF16, isOutput=False)
    ytp = nc.declare_dram_parameter("ytp", [128, MH * C], BF16, isOutput=True)

    with tile.TileContext(nc) as tc:
        with (
            tc.tile_pool(name="wsb", bufs=1) as wsb,
            tc.tile_pool(name="work", bufs=5) as work,
            tc.tile_pool(name="pmm", bufs=8, space="PSUM") as pmm,
        ):
            ab = wsb.tile([128, KH * AB], BF16, tag="ab")
            wb = wsb.tile([128, KF * WB], BF16, tag="wb")
            vds = wsb.tile([128, MH * R], BF16, tag="vds")
            g1s = wsb.tile([128, KR * C], BF16, tag="g1s")
            u1s = wsb.tile([128, KR * C], BF16, tag="u1s")
            d1s = wsb.tile([128, KR * C], BF16, tag="d1s")
            yt = wsb.tile([128, MH * C], BF16, tag="yt")
            warm = wsb.tile([128, 512], BF16, tag="warm")

            def ugc_k(k, m):
                o = k * AB + m * 128
                return ab[:, o:o + 128]

            def uuc_k(k, m):
                o = k * AB + R + m * 128
                return ab[:, o:o + 128]

            def xt_k(k, c0):
                o = k * AB + 2 * R + c0
                return ab[:, o:o + chunk]

            def vg_f(f, k):
                o = f * WB + k * 128
                return wb[:, o:o + 128]

            def vu_f(f, k):
                o = f * WB + R + k * 128
                return wb[:, o:o + 128]

            def udc_f(f, m):
                o = f * WB + 2 * R + m * 128
                return wb[:, o:o + 128]

            # --- PE warm-up: start the HAM activity window / p-state ramp
            # while the first input DMA is in flight. Vector memset is the
            # only dependency, so the PE starts right after queue entry.
            nc.vector.memset(warm[:], 0.0)
            wps = pmm.tile([128, 512], F32, tag="mm", name="wps")
            NWARM = 2
            for i in range(NWARM):
                nc.tensor.matmul(
                    wps[:], warm[:, :128], warm[:],
                    start=(i == 0), stop=(i == NWARM - 1),
                )

            # --- input DMAs: one serial ring (SP), in consumption order.
            # k=0 is split (weights, then x) so the first LDWEIGHTS unblocks
            # on a small early transfer instead of the whole block.
            nc.sync.dma_start(ab[:, 0:2 * R], abuf[:, 0:2 * R])
            nc.sync.dma_start(ab[:, 2 * R:AB], abuf[:, 2 * R:AB])
            for k in range(1, KH):
                nc.sync.dma_start(
                    ab[:, k * AB:(k + 1) * AB], abuf[:, k * AB:(k + 1) * AB]
                )
            for i in range(0, KF, 4):
                j = min(i + 4, KF)
                nc.sync.dma_start(
                    wb[:, i * WB:j * WB], wbuf[:, i * WB:j * WB]
                )
            nc.sync.dma_start(vds[:], vdp[:])

            # --- phase A: g1T/u1T [R, C] = Ug'/Uu'.T @ xT. k-outer with
            # 4*nch concurrent PSUM accumulators; compute starts on the
            # first k-block and paces the serial input DMA stream.
            psA = [
                pmm.tile([128, chunk], F32, tag="mm", name=f"psA_{n}_{t}_{m}")
                for n in range(nch) for t in range(2) for m in range(KR)
            ]
            for k in range(KH):
                for t, wfun in enumerate((ugc_k, uuc_k)):
                    for m in range(KR):
                        for n in range(nch):
                            nc.tensor.matmul(
                                psA[(n * 2 + t) * KR + m][:],
                                wfun(k, m),
                                xt_k(k, n * chunk),
                                start=(k == 0), stop=(k == KH - 1),
                            )
            # copies to bf16 staging, earliest-needed first (n=0 before n=1,
            # gate before up); split across vector / scalar engines.
            for n in range(nch):
                c0 = n * chunk
                for t, dst in enumerate((g1s, u1s)):
                    eng = nc.vector if t == 0 else nc.scalar
                    for m in range(KR):
                        src = psA[(n * 2 + t) * KR + m][:]
                        d = dst[:, m * C + c0:m * C + c0 + chunk]
                        if t == 0:
                            eng.tensor_copy(d, src)
                        else:
                            eng.activation(
                                d, src, mybir.ActivationFunctionType.Copy
                            )

            # --- phase B: f-loop over KF intermediate tiles. d1 matmuls for
            # f-1 are issued during f's gate/up matmuls (software pipeline)
            # so the silu*up DVE latency is hidden behind PE streaming.
            d1p = [
                pmm.tile([128, chunk], F32, tag="mm", name=f"d1p_{n}_{m}")
                for n in range(nch) for m in range(KR)
            ]

            af_prev = None

            def issue_d1(f, afs):
                for m in range(KR):
                    for n in range(nch):
                        nc.tensor.matmul(
                            d1p[n * KR + m][:], udc_f(f, m), afs[n][:],
                            start=(f == 0), stop=(f == KF - 1),
                        )

            for f in range(KF):
                gps = [
                    pmm.tile([128, chunk], F32, tag="mm", name=f"gps_{n}_{f}")
                    for n in range(nch)
                ]
                ups = [
                    pmm.tile([128, chunk], F32, tag="mm", name=f"ups_{n}_{f}")
                    for n in range(nch)
                ]
                for k in range(KR):
                    for n in range(nch):
                        c0 = n * chunk
                        nc.tensor.matmul(
                            gps[n][:], vg_f(f, k),
                            g1s[:, k * C + c0:k * C + c0 + chunk],
                            start=(k == 0), stop=(k == KR - 1),
                        )
                for k in range(KR):
                    for n in range(nch):
                        c0 = n * chunk
                        nc.tensor.matmul(
                            ups[n][:], vu_f(f, k),
                            u1s[:, k * C + c0:k * C + c0 + chunk],
                            start=(k == 0), stop=(k == KR - 1),
                        )
                if af_prev is not None:
                    issue_d1(f - 1, af_prev)
                afs = []
                for n in range(nch):
                    gsil = work.tile([128, chunk], BF16, tag="gsil")
                    nc.scalar.activation(
                        gsil[:], gps[n][:], mybir.ActivationFunctionType.Silu
                    )
                    af = work.tile([128, chunk], BF16, tag="af", name=f"af_{n}_{f}")
                    nc.vector.tensor_mul(af[:], gsil[:], ups[n][:])
                    afs.append(af)
                af_prev = afs
            issue_d1(KF - 1, af_prev)

            # d1 -> bf16 staging; n-outer so phase C's first (m, n=0)
            # accumulation pair unblocks earliest; vector/scalar in parallel.
            for n in range(nch):
                for k in range(KR):
                    c0 = n * chunk
                    src = d1p[n * KR + k][:]
                    d = d1s[:, k * C + c0:k * C + c0 + chunk]
                    if k % 2 == 0:
                        nc.vector.tensor_copy(d, src)
                    else:
                        nc.scalar.activation(
                            d, src, mybir.ActivationFunctionType.Copy
                        )

            # --- phase C: yT [H, C] = Vd.T @ d1T, staged to SBUF bf16,
            # one m-tile DMA issued as soon as that tile's copies land so
            # the write-back streams during phase C's remaining matmuls.
            out_engs = (nc.gpsimd, nc.scalar, nc.sync)
            for m in range(MH):
                ypsl = [
                    pmm.tile([128, chunk], F32, tag="mm", name=f"yps_{n}_{m}")
                    for n in range(nch)
                ]
                for k in range(KR):
                    for n in range(nch):
                        c0 = n * chunk
                        nc.tensor.matmul(
                            ypsl[n][:],
                            vds[:, m * R + k * 128:m * R + (k + 1) * 128],
                            d1s[:, k * C + c0:k * C + c0 + chunk],
                            start=(k == 0), stop=(k == KR - 1),
                        )
                for n in range(nch):
                    c0 = n * chunk
                    d = yt[:, m * C + c0:m * C + c0 + chunk]
                    # last m-tile's copies both on vector (faster), so the
                    # final DMA isn't gated on a busy scalar queue
                    if m == MH - 1 or (m * nch + n) % 2 == 0:
                        nc.vector.tensor_copy(d, ypsl[n][:])
                    else:
                        nc.scalar.activation(
                            d, ypsl[n][:], mybir.ActivationFunctionType.Copy
                        )
                # 3 output chunks: [0..3), [3..6), [6..8) — issue as soon as
                # the chunk's copies land, last (small) chunk on gpsimd
                if m == 2:
                    nc.gpsimd.dma_start(ytp[:, 0:3 * C], yt[:, 0:3 * C])
                elif m == 5:
                    nc.sync.dma_start(ytp[:, 3 * C:6 * C], yt[:, 3 * C:6 * C])
                elif m == MH - 1:
                    nc.gpsimd.dma_start(
                        ytp[:, 6 * C:MH * C], yt[:, 6 * C:MH * C]
                    )

    nc.finalize()
    return nc


def _pack_k(a, kt):
    """[kt*128, X] -> [128, kt, X] partition-tiled per k."""
    x = a.shape[1]
    return np.ascontiguousarray(a.reshape(kt, 128, x).transpose(1, 0, 2))


def _pack_fmajor(a, kt):
    """[kt*128, ft*128] -> [128, ft, kt*128]: f-major, k tiles adjacent."""
    ft = a.shape[1] // 128
    return np.ascontiguousarray(
        a.reshape(kt, 128, ft, 128).transpose(1, 2, 0, 3).reshape(128, ft, kt * 128)
    )


def kernel(hidden_states, gate_w, Ug, Cg, Vg, Uu, Cu, Vu, Ud, Cd, Vd):
    global LAST_RESULT
    hidden_states = np.asarray(hidden_states, dtype=np.float32)
    gate_w = np.asarray(gate_w, dtype=np.float32)
    b, s, h = hidden_states.shape
    x = hidden_states.reshape(-1, h)
    T = x.shape[0]

    # --- router (host; part of dispatch)
    logits = (x @ gate_w).astype(np.float64)
    lmax = logits.max(axis=-1, keepdims=True)
    p = np.exp(logits - lmax)
    p /= p.sum(axis=-1, keepdims=True)
    i1 = np.argmax(p, axis=-1)
    p1 = p[np.arange(T), i1]
    p_masked = p.copy()
    p_masked[np.arange(T), i1] = -np.inf
    i2 = np.argmax(p_masked, axis=-1)
    p2 = p[np.arange(T), i2]
    w1 = (p1 / (p1 + p2)).astype(np.float32)
    w2 = (p2 / (p1 + p2)).astype(np.float32)

    idx_e = []
    wgt_e = []
    for e in range(E):
        sel1 = np.nonzero(i1 == e)[0]
        sel2 = np.nonzero(i2 == e)[0]
        ids = np.concatenate([sel1, sel2])
        ws = np.concatenate([w1[sel1], w2[sel2]])
        idx_e.append(ids)
        wgt_e.append(ws)

    max_n = max(len(ids) for ids in idx_e)
    nch = max(1, -(-max_n // 512))
    chunk = -(-max_n // (nch * 4)) * 4
    C = nch * chunk

    key = (C, nch)
    if key not in _BUILD_CACHE:
        _BUILD_CACHE[key] = _build(C, nch)
    nc = _BUILD_CACHE[key]

    f32 = np.float32
    in_maps = []
    for e in range(E):
        ids = idx_e[e]
        xT = np.zeros((h, C), f32)
        xT[:, :len(ids)] = x[ids].T
        ugc = (Ug[e] @ Cg).astype(BF)
        uuc = (Uu[e] @ Cu).astype(BF)
        udc = (Ud[e] @ Cd).astype(BF)
        # abuf: per-k contiguous blocks [128, ugc_k | uuc_k | xt_k], flat
        abuf = np.ascontiguousarray(np.concatenate(
            [_pack_k(ugc, KH), _pack_k(uuc, KH), _pack_k(xT.astype(BF), KH)],
            axis=2,
        ).reshape(128, -1))  # [128, KH*AB]
        # wbuf: per-f blocks [vg_f | vu_f | udc_f], flat
        wbuf = np.ascontiguousarray(np.concatenate(
            [
                _pack_fmajor(np.asarray(Vg[e], BF), KR),
                _pack_fmajor(np.asarray(Vu[e], BF), KR),
                _pack_k(udc, KF),
            ],
            axis=2,
        ).reshape(128, -1))  # [128, KF*WB]
        in_maps.append({
            "abuf": abuf,
            "wbuf": wbuf,
            "vdp": np.ascontiguousarray(
                _pack_fmajor(np.asarray(Vd[e], BF), KR).reshape(128, -1)
            ),
        })

    res = run_bass_kernel_spmd(nc, in_maps, list(range(E)))
    LAST_RESULT = res

    out = np.zeros((T, h), f32)
    for e in range(E):
        ids = idx_e[e]
        ytp = np.asarray(res.results[e]["ytp"], dtype=f32)
        yT = ytp.reshape(128, MH, C).transpose(1, 0, 2).reshape(h, C)
        out[ids] += wgt_e[e][:, None] * yT[:, :len(ids)].T
    return out.reshape(b, s, h)


# revision 16
# speedup vs baseline: 1.0299x; 1.0299x over previous
"""Compressed MoE block on 8 Trainium2 NeuronCores.

Expert-parallel sharding: core e owns expert e. The router (tiny: T x H @
H x E) runs on host as part of dispatch; tokens are gathered per selected
expert (top-2), padded to a fixed capacity, and each core runs the full
factored FFN chain for its expert in token-transposed layout:

    g1T = Ug'(e).T @ xT          (Ug' = Ug @ Cg folded on host)
    gT  = Vg(e).T  @ g1T
    u1T = Uu'(e).T @ xT
    uT  = Vu(e).T  @ u1T
    aT  = silu(gT) * uT
    d1T = Ud'(e).T @ aT          (Ud' = Ud @ Cd)
    yT  = Vd(e).T  @ d1T

Everything on-chip is bf16 (PSUM accumulation fp32): halves HBM traffic
vs fp32 while staying well inside the accuracy budget. Weights stream in
consumption order on one DMA ring. Phase B is software-pipelined: the
down-proj (d1) matmuls for f-tile f are issued during f+1's gate/up
matmuls so the silu*up vector-engine latency never stalls the PE. Output
is staged to SBUF bf16 and written back in four 2-m-tile DMAs on
rotating queues so the drain overlaps phase C.
"""

import numpy as np
import ml_dtypes

import concourse.bacc as bacc
import concourse.mybir as mybir
import concourse.tile as tile
from concourse.bass_utils import run_bass_kernel_spmd

F32 = mybir.dt.float32
BF16 = mybir.dt.bfloat16
BF = ml_dtypes.bfloat16

E = 8
KTOP = 2
H = 1024
FF = 2816
R = 256
KH = H // 128    # 8
KR = R // 128    # 2
KF = FF // 128   # 22
MH = H // 128    # 8

_BUILD_CACHE = {}
LAST_RESULT = None


def _build(C, nch):
    """Build the per-core bass program for capacity C split into nch chunks."""
    chunk = C // nch
    AB = 2 * R + C      # per-k block in abuf: [ugc_k | uuc_k | xt_k]
    WB = 3 * R          # per-f block in wbuf: [vg_f | vu_f | udc_f]
    nc = bacc.Bacc()

    abuf = nc.declare_dram_parameter("abuf", [128, KH * AB], BF16, isOutput=False)
    wbuf = nc.declare_dram_parameter("wbuf", [128, KF * WB], BF16, isOutput=False)
    vdp = nc.declare_dram_parameter("vdp", [128, MH * R], BF16, isOutput=False)
    ytp = nc.declare_dram_parameter("ytp", [128, MH * C], BF16, isOutput=True)

    with tile.TileContext(nc) as tc:
        with (
            tc.tile_pool(name="wsb", bufs=1) as wsb,
            tc.tile_pool(name="work", bufs=5) as work,
            tc.tile_pool(name="pmm", bufs=8, space="PSUM") as pmm,
        ):
            ab = wsb.tile([128, KH * AB], BF16, tag="ab")
            wb = wsb.tile([128, KF * WB], BF16, tag="wb")
            vds = wsb.tile([128, MH * R], BF16, tag="vds")
            g1s = wsb.tile([128, KR * C], BF16, tag="g1s")
            u1s = wsb.tile([128, KR * C], BF16, tag="u1s")
            d1s = wsb.tile([128, KR * C], BF16, tag="d1s")
            yt = wsb.tile([128, MH * C], BF16, tag="yt")
            warm = wsb.tile([128, 512], BF16, tag="warm")

            def ugc_k(k, m):
                o = k * AB + m * 128
                return ab[:, o:o + 128]

            def uuc_k(k, m):
                o = k * AB + R + m * 128
                return ab[:, o:o + 128]

            def xt_k(k, c0):
                o = k * AB + 2 * R + c0
                return ab[:, o:o + chunk]

            def vg_f(f, k):
                o = f * WB + k * 128
                return wb[:, o:o + 128]

            def vu_f(f, k):
                o = f * WB + R + k * 128
                return wb[:, o:o + 128]

            def udc_f(f, m):
                o = f * WB + 2 * R + m * 128
                return wb[:, o:o + 128]

            # --- PE warm-up: start the HAM activity window / p-state ramp
            # while the first input DMA is in flight. Vector memset is the
            # only dependency, so the PE starts right after queue entry.
            nc.vector.memset(warm[:], 0.0)
            wps = pmm.tile([128, 512], F32, tag="mm", name="wps")
            NWARM = 2
            for i in range(NWARM):
                nc.tensor.matmul(
                    wps[:], warm[:, :128], warm[:],
                    start=(i == 0), stop=(i == NWARM - 1),
                )

            # --- input DMAs: one serial ring (SP), in consumption order.
            # k=0 is split (weights, then x) so the first LDWEIGHTS unblocks
            # on a small early transfer instead of the whole block.
            nc.scalar.dma_start(ab[:, 0:2 * R], abuf[:, 0:2 * R])
            nc.sync.dma_start(ab[:, 2 * R:AB], abuf[:, 2 * R:AB])
            for k in range(1, KH):
                nc.sync.dma_start(
                    ab[:, k * AB:(k + 1) * AB], abuf[:, k * AB:(k + 1) * AB]
                )
            for i in range(0, KF, 4):
                j = min(i + 4, KF)
                nc.sync.dma_start(
                    wb[:, i * WB:j * WB], wbuf[:, i * WB:j * WB]
                )
            nc.sync.dma_start(vds[:], vdp[:])

            # --- phase A: g1T/u1T [R, C] = Ug'/Uu'.T @ xT. k-outer with
            # 4*nch concurrent PSUM accumulators; compute starts on the
            # first k-block and paces the serial input DMA stream.
            psA = [
                pmm.tile([128, chunk], F32, tag="mm", name=f"psA_{n}_{t}_{m}")
                for n in range(nch) for t in range(2) for m in range(KR)
            ]
            for k in range(KH):
                for t, wfun in enumerate((ugc_k, uuc_k)):
                    for m in range(KR):
                        for n in range(nch):
                            nc.tensor.matmul(
                                psA[(n * 2 + t) * KR + m][:],
                                wfun(k, m),
                                xt_k(k, n * chunk),
                                start=(k == 0), stop=(k == KH - 1),
                            )
            # copies to bf16 staging, earliest-needed first (n=0 before n=1,
            # gate before up); split across vector / scalar engines.
            for n in range(nch):
                c0 = n * chunk
                for t, dst in enumerate((g1s, u1s)):
                    eng = nc.vector if t == 0 else nc.scalar
                    for m in range(KR):
                        src = psA[(n * 2 + t) * KR + m][:]
                        d = dst[:, m * C + c0:m * C + c0 + chunk]
                        if t == 0:
                            eng.tensor_copy(d, src)
                        else:
                            eng.activation(
                                d, src, mybir.ActivationFunctionType.Copy
                            )

            # --- phase B: f-loop over KF intermediate tiles. d1 matmuls for
            # f-1 are issued during f's gate/up matmuls (software pipeline)
            # so the silu*up DVE latency is hidden behind PE streaming.
            d1p = [
                pmm.tile([128, chunk], F32, tag="mm", name=f"d1p_{n}_{m}")
                for n in range(nch) for m in range(KR)
            ]

            af_prev = None

            def issue_d1(f, afs):
                for m in range(KR):
                    for n in range(nch):
                        nc.tensor.matmul(
                            d1p[n * KR + m][:], udc_f(f, m), afs[n][:],
                            start=(f == 0), stop=(f == KF - 1),
                        )

            for f in range(KF):
                gps = [
                    pmm.tile([128, chunk], F32, tag="mm", name=f"gps_{n}_{f}")
                    for n in range(nch)
                ]
                ups = [
                    pmm.tile([128, chunk], F32, tag="mm", name=f"ups_{n}_{f}")
                    for n in range(nch)
                ]
                for k in range(KR):
                    for n in range(nch):
                        c0 = n * chunk
                        nc.tensor.matmul(
                            gps[n][:], vg_f(f, k),
                            g1s[:, k * C + c0:k * C + c0 + chunk],
                            start=(k == 0), stop=(k == KR - 1),
                        )
                for k in range(KR):
                    for n in range(nch):
                        c0 = n * chunk
                        nc.tensor.matmul(
                            ups[n][:], vu_f(f, k),
                            u1s[:, k * C + c0:k * C + c0 + chunk],
                            start=(k == 0), stop=(k == KR - 1),
                        )
                if af_prev is not None:
                    issue_d1(f - 1, af_prev)
                afs = []
                for n in range(nch):
                    gsil = work.tile([128, chunk], BF16, tag="gsil")
                    nc.scalar.activation(
                        gsil[:], gps[n][:], mybir.ActivationFunctionType.Silu
                    )
                    af = work.tile([128, chunk], BF16, tag="af", name=f"af_{n}_{f}")
                    nc.vector.tensor_mul(af[:], gsil[:], ups[n][:])
                    afs.append(af)
                af_prev = afs
            issue_d1(KF - 1, af_prev)

            # d1 -> bf16 staging; n-outer so phase C's first (m, n=0)
            # accumulation pair unblocks earliest; vector/scalar in parallel.
            for n in range(nch):
                for k in range(KR):
                    c0 = n * chunk
                    src = d1p[n * KR + k][:]
                    d = d1s[:, k * C + c0:k * C + c0 + chunk]
                    if k % 2 == 0:
                        nc.vector.tensor_copy(d, src)
                    else:
                        nc.scalar.activation(
                            d, src, mybir.ActivationFunctionType.Copy
                        )

            # --- phase C: yT [H, C] = Vd.T @ d1T, staged to SBUF bf16,
            # one m-tile DMA issued as soon as that tile's copies land so
            # the write-back streams during phase C's remaining matmuls.
            out_engs = (nc.gpsimd, nc.scalar, nc.sync)
            for m in range(MH):
                ypsl = [
                    pmm.tile([128, chunk], F32, tag="mm", name=f"yps_{n}_{m}")
                    for n in range(nch)
                ]
                for k in range(KR):
                    for n in range(nch):
                        c0 = n * chunk
                        nc.tensor.matmul(
                            ypsl[n][:],
                            vds[:, m * R + k * 128:m * R + (k + 1) * 128],
                            d1s[:, k * C + c0:k * C + c0 + chunk],
                            start=(k == 0), stop=(k == KR - 1),
                        )
                for n in range(nch):
                    c0 = n * chunk
                    d = yt[:, m * C + c0:m * C + c0 + chunk]
                    # last m-tile's copies both on vector (faster), so the
                    # final DMA isn't gated on a busy scalar queue
                    if m == MH - 1 or (m * nch + n) % 2 == 0:
                        nc.vector.tensor_copy(d, ypsl[n][:])
                    else:
                        nc.scalar.activation(
                            d, ypsl[n][:], mybir.ActivationFunctionType.Copy
                        )
                # 3 output chunks: [0..3), [3..6), [6..8) — issue as soon as
                # the chunk's copies land. Hardware DMA rings only (sync /
                # scalar); gpsimd's software-dynamic path is ~2x slower.
                if m == 2:
                    nc.sync.dma_start(ytp[:, 0:3 * C], yt[:, 0:3 * C])
                elif m == 5:
                    nc.scalar.dma_start(ytp[:, 3 * C:6 * C], yt[:, 3 * C:6 * C])
                elif m == MH - 1:
                    nc.sync.dma_start(
                        ytp[:, 6 * C:MH * C], yt[:, 6 * C:MH * C]
                    )

    nc.finalize()
    return nc


def _pack_k(a, kt):
    """[kt*128, X] -> [128, kt, X] partition-tiled per k."""
    x = a.shape[1]
    return np.ascontiguousarray(a.reshape(kt, 128, x).transpose(1, 0, 2))


def _pack_fmajor(a, kt):
    """[kt*128, ft*128] -> [128, ft, kt*128]: f-major, k tiles adjacent."""
    ft = a.shape[1] // 128
    return np.ascontiguousarray(
        a.reshape(kt, 128, ft, 128).transpose(1, 2, 0, 3).reshape(128, ft, kt * 128)
    )


def kernel(hidden_states, gate_w, Ug, Cg, Vg, Uu, Cu, Vu, Ud, Cd, Vd):
    global LAST_RESULT
    hidden_states = np.asarray(hidden_states, dtype=np.float32)
    gate_w = np.asarray(gate_w, dtype=np.float32)
    b, s, h = hidden_states.shape
    x = hidden_states.reshape(-1, h)
    T = x.shape[0]

    # --- router (host; part of dispatch)
    logits = (x @ gate_w).astype(np.float64)
    lmax = logits.max(axis=-1, keepdims=True)
    p = np.exp(logits - lmax)
    p /= p.sum(axis=-1, keepdims=True)
    i1 = np.argmax(p, axis=-1)
    p1 = p[np.arange(T), i1]
    p_masked = p.copy()
    p_masked[np.arange(T), i1] = -np.inf
    i2 = np.argmax(p_masked, axis=-1)
    p2 = p[np.arange(T), i2]
    w1 = (p1 / (p1 + p2)).astype(np.float32)
    w2 = (p2 / (p1 + p2)).astype(np.float32)

    idx_e = []
    wgt_e = []
    for e in range(E):
        sel1 = np.nonzero(i1 == e)[0]
        sel2 = np.nonzero(i2 == e)[0]
        ids = np.concatenate([sel1, sel2])
        ws = np.concatenate([w1[sel1], w2[sel2]])
        idx_e.append(ids)
        wgt_e.append(ws)

    max_n = max(len(ids) for ids in idx_e)
    nch = max(1, -(-max_n // 512))
    chunk = -(-max_n // (nch * 4)) * 4
    C = nch * chunk

    key = (C, nch)
    if key not in _BUILD_CACHE:
        _BUILD_CACHE[key] = _build(C, nch)
    nc = _BUILD_CACHE[key]

    f32 = np.float32
    in_maps = []
    for e in range(E):
        ids = idx_e[e]
        xT = np.zeros((h, C), f32)
        xT[:, :len(ids)] = x[ids].T
        ugc = (Ug[e] @ Cg).astype(BF)
        uuc = (Uu[e] @ Cu).astype(BF)
        udc = (Ud[e] @ Cd).astype(BF)
        # abuf: per-k contiguous blocks [128, ugc_k | uuc_k | xt_k], flat
        abuf = np.ascontiguousarray(np.concatenate(
            [_pack_k(ugc, KH), _pack_k(uuc, KH), _pack_k(xT.astype(BF), KH)],
            axis=2,
        ).reshape(128, -1))  # [128, KH*AB]
        # wbuf: per-f blocks [vg_f | vu_f | udc_f], flat
        wbuf = np.ascontiguousarray(np.concatenate(
            [
                _pack_fmajor(np.asarray(Vg[e], BF), KR),
                _pack_fmajor(np.asarray(Vu[e], BF), KR),
                _pack_k(udc, KF),
            ],
            axis=2,
        ).reshape(128, -1))  # [128, KF*WB]
        in_maps.append({
            "abuf": abuf,
            "wbuf": wbuf,
            "vdp": np.ascontiguousarray(
                _pack_fmajor(np.asarray(Vd[e], BF), KR).reshape(128, -1)
            ),
        })

    res = run_bass_kernel_spmd(nc, in_maps, list(range(E)))
    LAST_RESULT = res

    out = np.zeros((T, h), f32)
    for e in range(E):
        ids = idx_e[e]
        ytp = np.asarray(res.results[e]["ytp"], dtype=f32)
        yT = ytp.reshape(128, MH, C).transpose(1, 0, 2).reshape(h, C)
        out[ids] += wgt_e[e][:, None] * yT[:, :len(ids)].T
    return out.reshape(b, s, h)
